# revision 13
# baseline (speedup 1.0000x reference)
"""3-layer GAT (PyG GATConv-style) on 8 Trainium2 NeuronCores.

Strategy (dst-partitioned, edge-sorted-by-dst):
  - Host: add self-loops, bin-pack nodes into 160 tiles of 128 balancing
    in-degree, relabel nodes, pad per-tile edge lists to CPT chunks of 128.
    Core c owns 20 consecutive dst tiles (2560 output rows).
  - Device, per layer:
      phase A: each core computes T_piece = [H | als | ald] for its 2560
               nodes via PE matmuls (rhs = [W | W@A] packed on host).
      phase B: AllGather T pieces -> full table T[20480, 320] in DRAM.
      phase C: per dst tile: dma_gather messages (256 fp32/edge) and
               alpha rows by src/dst; ee = exp(leaky(es+ed)); weight
               messages by ee per head; scatter into PSUM via one-hot
               S matmul (lhsT = S[e, dst_local]); epilogue divides by
               the accumulated denominator, adds bias, applies gelu.
      phase D: PE-transpose aggregated rows for the next layer's matmuls
               (or the final projection).
  - Softmax max-subtraction is dropped: |e| <= ~7 for this model family,
    exp() is safe in fp32 (verified against the reference numerically).
"""

import os

import numpy as np

import concourse.bacc as bacc
import concourse.bass as bass
import concourse.mybir as mybir
import concourse.tile as tile
from concourse.bass_utils import run_bass_kernel_spmd
from concourse.masks import make_identity

# problem constants (hardcoded per harness contract)
N, E = 20000, 320000
IN_C, HID, HEADS, OUT_C = 128, 64, 4, 32
HC = HID * HEADS  # 256
NEG_SLOPE = 0.2

NCORES = 8
P = 128
TPC = 20                # dst tiles per core
NT = NCORES * TPC       # 160 tiles
NPAD = NT * P           # 20480 padded nodes
TW = 320                # T table row width: [H(256) | als(4) | ald(4) | pad]
INERT = 200.0           # dst_local value for padded edge slots

f32 = mybir.dt.float32
i32 = mybir.dt.int32
i16 = mybir.dt.int16


# ----------------------------------------------------------------------------
# host preprocessing
# ----------------------------------------------------------------------------

def _preprocess(edge_index: np.ndarray):
    """Returns (perm, cpt, per-core packed arrays)."""
    import heapq

    src = np.concatenate([edge_index[0], np.arange(N, dtype=np.int64)])
    dst = np.concatenate([edge_index[1], np.arange(N, dtype=np.int64)])
    deg = np.bincount(dst, minlength=N)

    # bin-pack nodes into NT tiles of P slots, balancing total in-degree
    order = np.argsort(-deg, kind="stable")
    heap = [(0, t) for t in range(NT)]
    heapq.heapify(heap)
    slots_used = np.zeros(NT, np.int64)
    tile_of = np.empty(N, np.int64)
    slot_of = np.empty(N, np.int64)
    for n in order:
        while True:
            load, t = heapq.heappop(heap)
            if slots_used[t] < P:
                break
        tile_of[n] = t
        slot_of[n] = slots_used[t]
        slots_used[t] += 1
        heapq.heappush(heap, (load + int(deg[n]), t))

    perm = tile_of * P + slot_of  # old id -> new id

    # edges in new id space
    nsrc = perm[src]
    ndst = perm[dst]

    # one synthetic edge per padded node slot so denominators never hit 0
    pad_dst = []
    for t in range(NT):
        for s in range(slots_used[t], P):
            pad_dst.append(t * P + s)
    if pad_dst:
        pad_dst = np.asarray(pad_dst, np.int64)
        nsrc = np.concatenate([nsrc, np.zeros(len(pad_dst), np.int64)])
        ndst = np.concatenate([ndst, pad_dst])

    etile = ndst // P
    # sort edges by tile
    eorder = np.argsort(etile, kind="stable")
    nsrc, ndst, etile = nsrc[eorder], ndst[eorder], etile[eorder]
    counts = np.bincount(etile, minlength=NT)
    cpt = int(-(-counts.max() // P))  # ceil
    ni = cpt * P  # edge slots per tile

    # per-tile padded streams
    srcs = np.zeros((NT, ni), np.int64)          # gather row for msg + es
    dsts = np.zeros((NT, ni), np.int64)          # gather row for ed
    dloc = np.full((NT, ni), INERT, np.float32)  # dst_local within tile
    starts = np.concatenate([[0], np.cumsum(counts)])
    for t in range(NT):
        c = counts[t]
        sl = slice(starts[t], starts[t] + c)
        srcs[t, :c] = nsrc[sl]
        dsts[t, :c] = ndst[sl]
        dloc[t, :c] = (ndst[sl] - t * P).astype(np.float32)

    def wrap_idx(a):  # [ni] -> [128, ni//16] int16, 16-row pattern replicated
        w = a.reshape(ni // 16, 16).T.astype(np.int16)  # [16, ni//16]
        return np.tile(w, (8, 1))

    # per-core packing
    per_core = []
    for c in range(NCORES):
        ts = range(c * TPC, (c + 1) * TPC)
        sidx = np.concatenate([wrap_idx(srcs[t]) for t in ts], axis=1)
        didx = np.concatenate([wrap_idx(dsts[t]) for t in ts], axis=1)
        # dloc packed: [128, TPC*cpt], edge (t, j*128+p) -> [p, (t-c*TPC)*cpt+j]
        dl = np.concatenate(
            [dloc[t].reshape(cpt, P).T for t in ts], axis=1
        ).astype(np.float32)
        per_core.append((np.ascontiguousarray(sidx),
                         np.ascontiguousarray(didx),
                         np.ascontiguousarray(dl)))
    return perm, cpt, per_core


def _pack_weights(inputs):
    def blockdiag_a(a_s, a_d):
        A = np.zeros((HC, 8), np.float32)
        for h in range(HEADS):
            A[h * HID:(h + 1) * HID, h] = a_s[h]
            A[h * HID:(h + 1) * HID, 4 + h] = a_d[h]
        return A

    Rs = []
    for l, (W, a_s, a_d) in enumerate([
        (inputs["W0"], inputs["as0"], inputs["ad0"]),
        (inputs["W1"], inputs["as1"], inputs["ad1"]),
        (inputs["W2"], inputs["as2"], inputs["ad2"]),
    ]):
        W = np.asarray(W, np.float32)
        A = blockdiag_a(np.asarray(a_s, np.float32), np.asarray(a_d, np.float32))
        Rs.append(np.concatenate([W, W @ A], axis=1))  # [K, 264]
    BB = np.concatenate(
        [np.broadcast_to(np.asarray(inputs[f"b{l}"], np.float32), (P, HC))
         for l in range(3)], axis=1)  # [128, 768]
    BP = np.broadcast_to(np.asarray(inputs["bp"], np.float32), (P, OUT_C)).copy()
    RP = np.asarray(inputs["Wp"], np.float32)  # [256, 32]
    return Rs, np.ascontiguousarray(BB), BP, RP


# ----------------------------------------------------------------------------
# device program
# ----------------------------------------------------------------------------

_PROGRAM_CACHE = {}


def _build_program(cpt: int):
    n_layers = int(os.environ.get("GAT_LAYERS", "3"))
    no_exp = bool(int(os.environ.get("GAT_NOEXP", "0")))
    repeat = int(os.environ.get("GAT_REPEAT", "1"))
    _dbg = os.environ.get("GAT_DEBUG", "0")
    _agg = (os.environ.get("GAT_AGG_TILES", ""),
            os.environ.get("GAT_AGG_CHUNKS", ""))
    cache_key = (cpt, n_layers, no_exp, repeat, _dbg, _agg)
    if cache_key in _PROGRAM_CACHE:
        return _PROGRAM_CACHE[cache_key]
    agg_tiles = int(os.environ.get("GAT_AGG_TILES", str(TPC)))
    agg_chunks = int(os.environ.get("GAT_AGG_CHUNKS", str(cpt)))

    ni = cpt * P
    nc = bacc.Bacc("TRN2", target_bir_lowering=False, debug=False,
                   num_devices=NCORES)

    xT = nc.dram_tensor("xT", [P, TPC * P], f32, kind="ExternalInput")
    R0 = nc.dram_tensor("R0", [IN_C, 264], f32, kind="ExternalInput")
    R1 = nc.dram_tensor("R1", [HC, 264], f32, kind="ExternalInput")
    R2 = nc.dram_tensor("R2", [HC, 264], f32, kind="ExternalInput")
    RP = nc.dram_tensor("RP", [HC, OUT_C], f32, kind="ExternalInput")
    BB = nc.dram_tensor("BB", [P, 3 * HC], f32, kind="ExternalInput")
    BP = nc.dram_tensor("BP", [P, OUT_C], f32, kind="ExternalInput")
    SIDX = nc.dram_tensor("SIDX", [P, TPC * cpt * 8], i16, kind="ExternalInput")
    DIDX = nc.dram_tensor("DIDX", [P, TPC * cpt * 8], i16, kind="ExternalInput")
    DLOC = nc.dram_tensor("DLOC", [P, TPC * cpt], f32, kind="ExternalInput")
    Y = nc.dram_tensor("Y", [TPC * P, OUT_C], f32, kind="ExternalOutput")
    debug = bool(int(os.environ.get("GAT_DEBUG", "0")))
    if debug:
        DBGT = nc.dram_tensor("DBGT", [TPC * P, 264], f32, kind="ExternalOutput")
        DBGX = nc.dram_tensor("DBGX", [P, TPC * HC], f32, kind="ExternalOutput")
        DBGM = nc.dram_tensor("DBGM", [P, 8 * HC], f32, kind="ExternalOutput")
        DBGP = nc.dram_tensor("DBGP", [P, 260], f32, kind="ExternalOutput")

    with tile.TileContext(nc) as tc:
        with (
            tc.tile_pool(name="const", bufs=1) as cpool,
            tc.tile_pool(name="dram", bufs=1, space="DRAM") as dpool,
            tc.tile_pool(name="msg", bufs=2) as msgpool,
            tc.tile_pool(name="alv", bufs=4) as alpool,
            tc.tile_pool(name="sel", bufs=4) as spool,
            tc.tile_pool(name="work", bufs=3) as wpool,
            tc.tile_pool(name="ps", bufs=2, space="PSUM") as pspool,
            tc.tile_pool(name="psT", bufs=2, space="PSUM") as psTpool,
        ):
            # --- persistent constants ------------------------------------
            ident = cpool.tile([P, P], f32, tag="ident")
            make_identity(nc, ident[:])
            iotai = cpool.tile([P, P], i32, tag="iotai")
            nc.gpsimd.iota(iotai[:], pattern=[[1, P]], base=0,
                           channel_multiplier=0)
            iota = cpool.tile([P, P], f32, tag="iota")
            nc.vector.tensor_copy(out=iota[:], in_=iotai[:])

            xT_sb = cpool.tile([P, TPC * P], f32, tag="xT")
            nc.sync.dma_start(out=xT_sb[:], in_=xT[:])
            R0_sb = cpool.tile([P, 264], f32, tag="R0")
            nc.sync.dma_start(out=R0_sb[:], in_=R0[:])
            R1_sb = cpool.tile([P, 2, 264], f32, tag="R1")
            nc.sync.dma_start(out=R1_sb[:, 0, :], in_=R1[0:P, :])
            nc.sync.dma_start(out=R1_sb[:, 1, :], in_=R1[P:HC, :])
            R2_sb = cpool.tile([P, 2, 264], f32, tag="R2")
            nc.sync.dma_start(out=R2_sb[:, 0, :], in_=R2[0:P, :])
            nc.sync.dma_start(out=R2_sb[:, 1, :], in_=R2[P:HC, :])
            RP_sb = cpool.tile([P, 2, OUT_C], f32, tag="RP")
            nc.sync.dma_start(out=RP_sb[:, 0, :], in_=RP[0:P, :])
            nc.sync.dma_start(out=RP_sb[:, 1, :], in_=RP[P:HC, :])
            BB_sb = cpool.tile([P, 3 * HC], f32, tag="BB")
            nc.sync.dma_start(out=BB_sb[:], in_=BB[:])
            BP_sb = cpool.tile([P, OUT_C], f32, tag="BP")
            nc.sync.dma_start(out=BP_sb[:], in_=BP[:])
            sidx_sb = cpool.tile([P, TPC * cpt * 8], i16, tag="sidx")
            nc.sync.dma_start(out=sidx_sb[:], in_=SIDX[:])
            didx_sb = cpool.tile([P, TPC * cpt * 8], i16, tag="didx")
            nc.sync.dma_start(out=didx_sb[:], in_=DIDX[:])
            dloc_sb = cpool.tile([P, TPC * cpt], f32, tag="dloc")
            nc.sync.dma_start(out=dloc_sb[:], in_=DLOC[:])

            # persistent layer state
            XT_sb = cpool.tile([P, 2, TPC * P], f32, tag="XT")  # transposed input
            XN_sb = cpool.tile([P, TPC * HC], f32, tag="XN")    # aggregated rows

            Tin = dpool.tile([TPC * P, TW], f32, tag="Tin")
            # Shared addr space is required for performance: a gather whose
            # source is a non-Shared collective output hits a ~150s sync
            # pathology on this runtime (measured in probe4). A Shared tile
            # admits only one writer, so allocate one per layer.
            Tfulls = [dpool.tile([NPAD, TW], f32, tag=f"Tfull{l}",
                                 addr_space="Shared", name=f"Tfull{l}")
                      for l in range(3 * repeat)]

            R_blocks = [[R0_sb[:, :]], [R1_sb[:, 0, :], R1_sb[:, 1, :]],
                        [R2_sb[:, 0, :], R2_sb[:, 1, :]]]

            for rep, l in [(r, ll) for r in range(repeat)
                           for ll in range(n_layers)]:
                Tfull = Tfulls[rep * 3 + l]
                # --- phase A: T piece = [H | als | ald] -------------------
                for t in range(TPC):
                    psT = psTpool.tile([P, 264], f32, tag="psT")
                    if l == 0:
                        lhs = [xT_sb[:, t * P:(t + 1) * P]]
                    else:
                        lhs = [XT_sb[:, 0, t * P:(t + 1) * P],
                               XT_sb[:, 1, t * P:(t + 1) * P]]
                    for k, (lh, rh) in enumerate(zip(lhs, R_blocks[l])):
                        nc.tensor.matmul(out=psT[:], lhsT=lh, rhs=rh,
                                         start=(k == 0), stop=(k == len(lhs) - 1))
                    Tt = wpool.tile([P, TW], f32, tag="Tt")
                    nc.vector.tensor_copy(out=Tt[:, 0:264], in_=psT[:])
                    nc.sync.dma_start(out=Tin[t * P:(t + 1) * P, 0:264],
                                      in_=Tt[:, 0:264])
                    if debug and l == 0:
                        nc.sync.dma_start(out=DBGT[t * P:(t + 1) * P, :],
                                          in_=Tt[:, 0:264])

                # --- phase B: AllGather ----------------------------------
                nc.gpsimd.collective_compute(
                    "AllGather",
                    mybir.AluOpType.bypass,
                    ins=[Tin[:]],
                    outs=[Tfull[:]],
                    replica_groups=[list(range(NCORES))],
                )

                # --- phase C: aggregation over own dst tiles -------------
                # dma_gather crashes above ~1024 indices per call (measured:
                # 1024 ok, 2048 crashes), so gather in groups of <=8 chunks.
                GMAX = 8
                groups = [(a, min(a + GMAX, agg_chunks))
                          for a in range(0, agg_chunks, GMAX)]
                for t in range(agg_tiles):
                    i0 = t * cpt * 8
                    # separate PSUM tiles: start=True clears the whole PSUM
                    # bank, so numerator and denominator accumulation streams
                    # must not share a bank (measured in probe7).
                    psn = pspool.tile([P, HC], f32, tag="psn")
                    psd = pspool.tile([P, 4], f32, tag="psd")
                    for gi, (ga, gb) in enumerate(groups):
                        g = gb - ga
                        gni = g * P
                        msg = msgpool.tile([P, GMAX, HC], f32, tag="msg")
                        nc.gpsimd.dma_gather(
                            out_ap=msg[:, 0:g, :], in_ap=Tfull[:, 0:HC],
                            idxs_ap=sidx_sb[:, i0 + ga * 8:i0 + gb * 8],
                            num_idxs=gni, num_idxs_reg=gni,
                            elem_size=HC, elem_step=TW)
                        esb = alpool.tile([P, GMAX, 64], f32, tag="esb")
                        nc.gpsimd.dma_gather(
                            out_ap=esb[:, 0:g, :], in_ap=Tfull[:, HC:TW],
                            idxs_ap=sidx_sb[:, i0 + ga * 8:i0 + gb * 8],
                            num_idxs=gni, num_idxs_reg=gni,
                            elem_size=64, elem_step=TW)
                        edb = alpool.tile([P, GMAX, 64], f32, tag="edb")
                        nc.gpsimd.dma_gather(
                            out_ap=edb[:, 0:g, :], in_ap=Tfull[:, HC:TW],
                            idxs_ap=didx_sb[:, i0 + ga * 8:i0 + gb * 8],
                            num_idxs=gni, num_idxs_reg=gni,
                            elem_size=64, elem_step=TW)

                        if debug and l == 0 and t == 0 and gi == 0:
                            dbgm_sb = wpool.tile([P, 8 * HC], f32, tag="dbgm")
                            nc.vector.tensor_copy(out=dbgm_sb[:],
                                                  in_=msg[:].rearrange("p a b -> p (a b)"))
                            nc.sync.dma_start(out=DBGM[:], in_=dbgm_sb[:])
                        # ee = exp(leaky_relu(es + ed))
                        ee = wpool.tile([P, GMAX, 4], f32, tag="ee")
                        tmp = wpool.tile([P, GMAX, 4], f32, tag="tmp")
                        nc.vector.tensor_add(out=ee[:, 0:g, :],
                                             in0=esb[:, 0:g, 0:4],
                                             in1=edb[:, 0:g, 4:8])
                        nc.vector.tensor_scalar_mul(tmp[:, 0:g, :],
                                                    ee[:, 0:g, :], NEG_SLOPE)
                        nc.vector.tensor_max(out=ee[:, 0:g, :],
                                             in0=ee[:, 0:g, :],
                                             in1=tmp[:, 0:g, :])
                        if no_exp:
                            nc.vector.memset(ee[:, 0:g, :], 1.0)
                        else:
                            nc.scalar.activation(
                                out=ee[:, 0:g, :], in_=ee[:, 0:g, :],
                                func=mybir.ActivationFunctionType.Exp)

                        for j in range(g):
                            jj = ga + j
                            S = spool.tile([P, P], f32, tag="S")
                            nc.vector.tensor_tensor(
                                out=S[:],
                                in0=dloc_sb[:, t * cpt + jj:t * cpt + jj + 1]
                                    .to_broadcast([P, P]),
                                in1=iota[:],
                                op=mybir.AluOpType.is_equal)
                            mj = msg[:, j, :].rearrange("p (h c) -> p h c",
                                                        h=HEADS)
                            nc.vector.tensor_mul(
                                out=mj, in0=mj,
                                in1=ee[:, j, :].to_broadcast([P, HEADS, HID]))
                            first = (jj == 0)
                            last = (jj == agg_chunks - 1)
                            nc.tensor.matmul(out=psn[:], lhsT=S[:],
                                             rhs=msg[:, j, :],
                                             start=first, stop=last)
                            nc.tensor.matmul(out=psd[:], lhsT=S[:],
                                             rhs=ee[:, j, :],
                                             start=first, stop=last)

                    if debug and l == 0 and t == 0:
                        dbgp_sb = wpool.tile([P, 260], f32, tag="dbgp")
                        nc.vector.tensor_copy(out=dbgp_sb[:, 0:HC], in_=psn[:])
                        nc.vector.tensor_copy(out=dbgp_sb[:, HC:260], in_=psd[:])
                        nc.sync.dma_start(out=DBGP[:], in_=dbgp_sb[:])
                    # epilogue: divide, bias, gelu
                    rcp = wpool.tile([P, 4], f32, tag="rcp")
                    nc.vector.reciprocal(out=rcp[:], in_=psd[:])
                    xn = XN_sb[:, t * HC:(t + 1) * HC]
                    nc.vector.tensor_mul(
                        out=xn.rearrange("p (h c) -> p h c", h=HEADS),
                        in0=psn[:].rearrange("p (h c) -> p h c", h=HEADS),
                        in1=rcp[:].to_broadcast([P, HEADS, HID]))
                    nc.vector.tensor_add(out=xn, in0=xn,
                                         in1=BB_sb[:, l * HC:(l + 1) * HC])
                    nc.scalar.activation(out=xn, in_=xn,
                                         func=mybir.ActivationFunctionType.Gelu)

                if debug and l == n_layers - 1:
                    dbgx_sb = wpool.tile([P, TPC * HC], f32, tag="dbgx")
                    nc.vector.tensor_copy(out=dbgx_sb[:], in_=XN_sb[:])
                    nc.sync.dma_start(out=DBGX[:], in_=dbgx_sb[:])

                # --- phase D: transpose for next matmuls ------------------
                for t in range(TPC):
                    for k in range(2):
                        pstr = psTpool.tile([P, P], f32, tag="psT")
                        nc.tensor.transpose(
                            out=pstr[:],
                            in_=XN_sb[:, t * HC + k * P:t * HC + (k + 1) * P],
                            identity=ident[:])
                        nc.vector.tensor_copy(
                            out=XT_sb[:, k, t * P:(t + 1) * P], in_=pstr[:])

            # --- final projection ----------------------------------------
            for t in range(TPC):
                psp = psTpool.tile([P, OUT_C], f32, tag="psT")
                nc.tensor.matmul(out=psp[:], lhsT=XT_sb[:, 0, t * P:(t + 1) * P],
                                 rhs=RP_sb[:, 0, :], start=True, stop=False)
                nc.tensor.matmul(out=psp[:], lhsT=XT_sb[:, 1, t * P:(t + 1) * P],
                                 rhs=RP_sb[:, 1, :], start=False, stop=True)
                yt = wpool.tile([P, OUT_C], f32, tag="yt")
                nc.vector.tensor_add(out=yt[:], in0=psp[:], in1=BP_sb[:])
                nc.sync.dma_start(out=Y[t * P:(t + 1) * P, :], in_=yt[:])

    nc.compile()
    _PROGRAM_CACHE[cache_key] = nc
    return nc


# ----------------------------------------------------------------------------
# entry point
# ----------------------------------------------------------------------------

def _make_in_maps(inputs, perm, cpt, per_core):
    x = np.asarray(inputs["x"], np.float32)
    x_pad = np.zeros((NPAD, IN_C), np.float32)
    x_pad[perm] = x
    xT_all = np.ascontiguousarray(x_pad.T)  # [128, 20480]

    Rs, BBa, BPa, RPa = _pack_weights(inputs)

    in_maps = []
    for c in range(NCORES):
        sidx, didx, dl = per_core[c]
        in_maps.append({
            "xT": np.ascontiguousarray(
                xT_all[:, c * TPC * P:(c + 1) * TPC * P]),
            "R0": Rs[0], "R1": Rs[1], "R2": Rs[2],
            "RP": RPa, "BB": BBa, "BP": BPa,
            "SIDX": sidx, "DIDX": didx, "DLOC": dl,
        })
    return in_maps


def kernel(**inputs) -> np.ndarray:
    edge_index = np.asarray(inputs["edge_index"])
    perm, cpt, per_core = _preprocess(edge_index)
    nc = _build_program(cpt)
    in_maps = _make_in_maps(inputs, perm, cpt, per_core)
    res = run_bass_kernel_spmd(nc, in_maps, core_ids=list(range(NCORES)))
    y_new = np.concatenate([res.results[c]["Y"] for c in range(NCORES)], axis=0)
    return np.ascontiguousarray(y_new[perm]).astype(np.float32)


# revision 15
# speedup vs baseline: 1.1167x; 1.1167x over previous
"""3-layer GAT (PyG GATConv-style) on 8 Trainium2 NeuronCores.

Strategy (dst-partitioned, edge-sorted-by-dst):
  - Host: add self-loops, bin-pack nodes into 160 tiles of 128 balancing
    in-degree, relabel nodes, pad per-tile edge lists to CPT chunks of 128.
    Core c owns 20 consecutive dst tiles (2560 output rows).
  - Device, per layer:
      phase A: each core computes T_piece = [H | als | ald] for its 2560
               nodes via PE matmuls (rhs = [W | W@A] packed on host).
      phase B: AllGather T pieces -> full table T[20480, 320] in DRAM.
      phase C: per dst tile: dma_gather messages (256 fp32/edge) and
               alpha rows by src/dst; ee = exp(leaky(es+ed)); weight
               messages by ee per head; scatter into PSUM via one-hot
               S matmul (lhsT = S[e, dst_local]); epilogue divides by
               the accumulated denominator, adds bias, applies gelu.
      phase D: PE-transpose aggregated rows for the next layer's matmuls
               (or the final projection).
  - Softmax max-subtraction is dropped: |e| <= ~7 for this model family,
    exp() is safe in fp32 (verified against the reference numerically).
"""

import os

import numpy as np

import concourse.bacc as bacc
import concourse.bass as bass
import concourse.mybir as mybir
import concourse.tile as tile
from concourse.bass_utils import run_bass_kernel_spmd
from concourse.masks import make_identity

# problem constants (hardcoded per harness contract)
N, E = 20000, 320000
IN_C, HID, HEADS, OUT_C = 128, 64, 4, 32
HC = HID * HEADS  # 256
NEG_SLOPE = 0.2

NCORES = 8
P = 128
TPC = 20                # dst tiles per core
NT = NCORES * TPC       # 160 tiles
NPAD = NT * P           # 20480 padded nodes
TW = 320                # T table row width: [H(256) | als(4) | ald(4) | pad]
INERT = 200.0           # dst_local value for padded edge slots

f32 = mybir.dt.float32
i32 = mybir.dt.int32
i16 = mybir.dt.int16


# ----------------------------------------------------------------------------
# host preprocessing
# ----------------------------------------------------------------------------

def _preprocess(edge_index: np.ndarray):
    """Returns (perm, cpt, per-core packed arrays)."""
    import heapq

    src = np.concatenate([edge_index[0], np.arange(N, dtype=np.int64)])
    dst = np.concatenate([edge_index[1], np.arange(N, dtype=np.int64)])
    deg = np.bincount(dst, minlength=N)

    # bin-pack nodes into NT tiles of P slots, balancing total in-degree
    order = np.argsort(-deg, kind="stable")
    heap = [(0, t) for t in range(NT)]
    heapq.heapify(heap)
    slots_used = np.zeros(NT, np.int64)
    tile_of = np.empty(N, np.int64)
    slot_of = np.empty(N, np.int64)
    for n in order:
        while True:
            load, t = heapq.heappop(heap)
            if slots_used[t] < P:
                break
        tile_of[n] = t
        slot_of[n] = slots_used[t]
        slots_used[t] += 1
        heapq.heappush(heap, (load + int(deg[n]), t))

    perm = tile_of * P + slot_of  # old id -> new id

    # edges in new id space
    nsrc = perm[src]
    ndst = perm[dst]

    # one synthetic edge per padded node slot so denominators never hit 0
    pad_dst = []
    for t in range(NT):
        for s in range(slots_used[t], P):
            pad_dst.append(t * P + s)
    if pad_dst:
        pad_dst = np.asarray(pad_dst, np.int64)
        nsrc = np.concatenate([nsrc, np.zeros(len(pad_dst), np.int64)])
        ndst = np.concatenate([ndst, pad_dst])

    etile = ndst // P
    # sort edges by tile
    eorder = np.argsort(etile, kind="stable")
    nsrc, ndst, etile = nsrc[eorder], ndst[eorder], etile[eorder]
    counts = np.bincount(etile, minlength=NT)
    cpt = int(-(-counts.max() // P))  # ceil
    ni = cpt * P  # edge slots per tile

    # per-tile padded streams
    srcs = np.zeros((NT, ni), np.int64)          # gather row for msg + es
    dsts = np.zeros((NT, ni), np.int64)          # gather row for ed
    dloc = np.full((NT, ni), INERT, np.float32)  # dst_local within tile
    starts = np.concatenate([[0], np.cumsum(counts)])
    for t in range(NT):
        c = counts[t]
        sl = slice(starts[t], starts[t] + c)
        srcs[t, :c] = nsrc[sl]
        dsts[t, :c] = ndst[sl]
        dloc[t, :c] = (ndst[sl] - t * P).astype(np.float32)

    def wrap_idx(a):  # [ni] -> [128, ni//16] int16, 16-row pattern replicated
        w = a.reshape(ni // 16, 16).T.astype(np.int16)  # [16, ni//16]
        return np.tile(w, (8, 1))

    # per-core packing
    per_core = []
    for c in range(NCORES):
        ts = range(c * TPC, (c + 1) * TPC)
        sidx = np.concatenate([wrap_idx(srcs[t]) for t in ts], axis=1)
        didx = np.concatenate([wrap_idx(dsts[t]) for t in ts], axis=1)
        # dloc packed: [128, TPC*cpt], edge (t, j*128+p) -> [p, (t-c*TPC)*cpt+j]
        dl = np.concatenate(
            [dloc[t].reshape(cpt, P).T for t in ts], axis=1
        ).astype(np.float32)
        per_core.append((np.ascontiguousarray(sidx),
                         np.ascontiguousarray(didx),
                         np.ascontiguousarray(dl)))
    return perm, cpt, per_core


def _pack_weights(inputs):
    def blockdiag_a(a_s, a_d):
        A = np.zeros((HC, 8), np.float32)
        for h in range(HEADS):
            A[h * HID:(h + 1) * HID, h] = a_s[h]
            A[h * HID:(h + 1) * HID, 4 + h] = a_d[h]
        return A

    Rs = []
    for l, (W, a_s, a_d) in enumerate([
        (inputs["W0"], inputs["as0"], inputs["ad0"]),
        (inputs["W1"], inputs["as1"], inputs["ad1"]),
        (inputs["W2"], inputs["as2"], inputs["ad2"]),
    ]):
        W = np.asarray(W, np.float32)
        A = blockdiag_a(np.asarray(a_s, np.float32), np.asarray(a_d, np.float32))
        Rs.append(np.concatenate([W, W @ A], axis=1))  # [K, 264]
    BB = np.concatenate(
        [np.broadcast_to(np.asarray(inputs[f"b{l}"], np.float32), (P, HC))
         for l in range(3)], axis=1)  # [128, 768]
    BP = np.broadcast_to(np.asarray(inputs["bp"], np.float32), (P, OUT_C)).copy()
    RP = np.asarray(inputs["Wp"], np.float32)  # [256, 32]
    return Rs, np.ascontiguousarray(BB), BP, RP


# ----------------------------------------------------------------------------
# device program
# ----------------------------------------------------------------------------

_PROGRAM_CACHE = {}


def _build_program(cpt: int):
    n_layers = int(os.environ.get("GAT_LAYERS", "3"))
    no_exp = bool(int(os.environ.get("GAT_NOEXP", "0")))
    repeat = int(os.environ.get("GAT_REPEAT", "1"))
    _dbg = os.environ.get("GAT_DEBUG", "0")
    _agg = (os.environ.get("GAT_AGG_TILES", ""),
            os.environ.get("GAT_AGG_CHUNKS", ""))
    cache_key = (cpt, n_layers, no_exp, repeat, _dbg, _agg)
    if cache_key in _PROGRAM_CACHE:
        return _PROGRAM_CACHE[cache_key]
    agg_tiles = int(os.environ.get("GAT_AGG_TILES", str(TPC)))
    agg_chunks = int(os.environ.get("GAT_AGG_CHUNKS", str(cpt)))

    ni = cpt * P
    nc = bacc.Bacc("TRN2", target_bir_lowering=False, debug=False,
                   num_devices=NCORES)

    xT = nc.dram_tensor("xT", [P, TPC * P], f32, kind="ExternalInput")
    R0 = nc.dram_tensor("R0", [IN_C, 264], f32, kind="ExternalInput")
    R1 = nc.dram_tensor("R1", [HC, 264], f32, kind="ExternalInput")
    R2 = nc.dram_tensor("R2", [HC, 264], f32, kind="ExternalInput")
    RP = nc.dram_tensor("RP", [HC, OUT_C], f32, kind="ExternalInput")
    BB = nc.dram_tensor("BB", [P, 3 * HC], f32, kind="ExternalInput")
    BP = nc.dram_tensor("BP", [P, OUT_C], f32, kind="ExternalInput")
    SIDX = nc.dram_tensor("SIDX", [P, TPC * cpt * 8], i16, kind="ExternalInput")
    DIDX = nc.dram_tensor("DIDX", [P, TPC * cpt * 8], i16, kind="ExternalInput")
    DLOC = nc.dram_tensor("DLOC", [P, TPC * cpt], f32, kind="ExternalInput")
    Y = nc.dram_tensor("Y", [TPC * P, OUT_C], f32, kind="ExternalOutput")
    debug = bool(int(os.environ.get("GAT_DEBUG", "0")))
    if debug:
        DBGT = nc.dram_tensor("DBGT", [TPC * P, 264], f32, kind="ExternalOutput")
        DBGX = nc.dram_tensor("DBGX", [P, TPC * HC], f32, kind="ExternalOutput")
        DBGM = nc.dram_tensor("DBGM", [P, 8 * HC], f32, kind="ExternalOutput")
        DBGP = nc.dram_tensor("DBGP", [P, 260], f32, kind="ExternalOutput")

    with tile.TileContext(nc) as tc:
        with (
            tc.tile_pool(name="const", bufs=1) as cpool,
            tc.tile_pool(name="dram", bufs=1, space="DRAM") as dpool,
            tc.tile_pool(name="msg", bufs=2) as msgpool,
            tc.tile_pool(name="alv", bufs=4) as alpool,
            tc.tile_pool(name="sel", bufs=4) as spool,
            tc.tile_pool(name="work", bufs=3) as wpool,
            tc.tile_pool(name="ps", bufs=2, space="PSUM") as pspool,
            tc.tile_pool(name="psT", bufs=2, space="PSUM") as psTpool,
        ):
            # --- persistent constants ------------------------------------
            ident = cpool.tile([P, P], f32, tag="ident")
            make_identity(nc, ident[:])
            iotai = cpool.tile([P, P], i32, tag="iotai")
            nc.gpsimd.iota(iotai[:], pattern=[[1, P]], base=0,
                           channel_multiplier=0)
            iota = cpool.tile([P, P], f32, tag="iota")
            nc.vector.tensor_copy(out=iota[:], in_=iotai[:])

            xT_sb = cpool.tile([P, TPC * P], f32, tag="xT")
            nc.sync.dma_start(out=xT_sb[:], in_=xT[:])
            R0_sb = cpool.tile([P, 264], f32, tag="R0")
            nc.sync.dma_start(out=R0_sb[:], in_=R0[:])
            R1_sb = cpool.tile([P, 2, 264], f32, tag="R1")
            nc.sync.dma_start(out=R1_sb[:, 0, :], in_=R1[0:P, :])
            nc.sync.dma_start(out=R1_sb[:, 1, :], in_=R1[P:HC, :])
            R2_sb = cpool.tile([P, 2, 264], f32, tag="R2")
            nc.sync.dma_start(out=R2_sb[:, 0, :], in_=R2[0:P, :])
            nc.sync.dma_start(out=R2_sb[:, 1, :], in_=R2[P:HC, :])
            RP_sb = cpool.tile([P, 2, OUT_C], f32, tag="RP")
            nc.sync.dma_start(out=RP_sb[:, 0, :], in_=RP[0:P, :])
            nc.sync.dma_start(out=RP_sb[:, 1, :], in_=RP[P:HC, :])
            BB_sb = cpool.tile([P, 3 * HC], f32, tag="BB")
            nc.sync.dma_start(out=BB_sb[:], in_=BB[:])
            BP_sb = cpool.tile([P, OUT_C], f32, tag="BP")
            nc.sync.dma_start(out=BP_sb[:], in_=BP[:])
            sidx_sb = cpool.tile([P, TPC * cpt * 8], i16, tag="sidx")
            nc.sync.dma_start(out=sidx_sb[:], in_=SIDX[:])
            didx_sb = cpool.tile([P, TPC * cpt * 8], i16, tag="didx")
            nc.sync.dma_start(out=didx_sb[:], in_=DIDX[:])
            dloc_sb = cpool.tile([P, TPC * cpt], f32, tag="dloc")
            nc.sync.dma_start(out=dloc_sb[:], in_=DLOC[:])

            # persistent layer state
            XT_sb = cpool.tile([P, 2, TPC * P], f32, tag="XT")  # transposed input
            XN_sb = cpool.tile([P, TPC * HC], f32, tag="XN")    # aggregated rows

            Tin = dpool.tile([TPC * P, TW], f32, tag="Tin")
            # Shared addr space is required for performance: a gather whose
            # source is a non-Shared collective output hits a ~150s sync
            # pathology on this runtime (measured in probe4). A Shared tile
            # admits only one writer, so allocate one per layer.
            Tfulls = [dpool.tile([NPAD, TW], f32, tag=f"Tfull{l}",
                                 addr_space="Shared", name=f"Tfull{l}")
                      for l in range(3 * repeat)]

            R_blocks = [[R0_sb[:, :]], [R1_sb[:, 0, :], R1_sb[:, 1, :]],
                        [R2_sb[:, 0, :], R2_sb[:, 1, :]]]

            for rep, l in [(r, ll) for r in range(repeat)
                           for ll in range(n_layers)]:
                Tfull = Tfulls[rep * 3 + l]
                # --- phase A: T piece = [H | als | ald] -------------------
                for t in range(TPC):
                    psT = psTpool.tile([P, 264], f32, tag="psT")
                    if l == 0:
                        lhs = [xT_sb[:, t * P:(t + 1) * P]]
                    else:
                        lhs = [XT_sb[:, 0, t * P:(t + 1) * P],
                               XT_sb[:, 1, t * P:(t + 1) * P]]
                    for k, (lh, rh) in enumerate(zip(lhs, R_blocks[l])):
                        nc.tensor.matmul(out=psT[:], lhsT=lh, rhs=rh,
                                         start=(k == 0), stop=(k == len(lhs) - 1))
                    Tt = wpool.tile([P, TW], f32, tag="Tt")
                    nc.vector.tensor_copy(out=Tt[:, 0:264], in_=psT[:])
                    nc.sync.dma_start(out=Tin[t * P:(t + 1) * P, 0:264],
                                      in_=Tt[:, 0:264])
                    if debug and l == 0:
                        nc.sync.dma_start(out=DBGT[t * P:(t + 1) * P, :],
                                          in_=Tt[:, 0:264])

                # --- phase B: AllGather ----------------------------------
                nc.gpsimd.collective_compute(
                    "AllGather",
                    mybir.AluOpType.bypass,
                    ins=[Tin[:]],
                    outs=[Tfull[:]],
                    replica_groups=[list(range(NCORES))],
                )

                # --- phase C: aggregation over own dst tiles -------------
                # dma_gather crashes above ~1024 indices per call (measured:
                # 1024 ok, 2048 crashes), so gather in groups of <=8 chunks.
                GMAX = 8
                groups = [(a, min(a + GMAX, agg_chunks))
                          for a in range(0, agg_chunks, GMAX)]
                for t in range(agg_tiles):
                    i0 = t * cpt * 8
                    # separate PSUM tiles: start=True clears the whole PSUM
                    # bank, so numerator and denominator accumulation streams
                    # must not share a bank (measured in probe7).
                    psn = pspool.tile([P, HC], f32, tag="psn")
                    psd = pspool.tile([P, 4], f32, tag="psd")
                    for gi, (ga, gb) in enumerate(groups):
                        g = gb - ga
                        gni = g * P
                        msg = msgpool.tile([P, GMAX, TW], f32, tag="msg")
                        nc.gpsimd.dma_gather(
                            out_ap=msg[:, 0:g, :], in_ap=Tfull[:],
                            idxs_ap=sidx_sb[:, i0 + ga * 8:i0 + gb * 8],
                            num_idxs=gni, num_idxs_reg=gni,
                            elem_size=TW, elem_step=TW)
                        edb = alpool.tile([P, GMAX, 64], f32, tag="edb")
                        nc.gpsimd.dma_gather(
                            out_ap=edb[:, 0:g, :], in_ap=Tfull[:, HC:TW],
                            idxs_ap=didx_sb[:, i0 + ga * 8:i0 + gb * 8],
                            num_idxs=gni, num_idxs_reg=gni,
                            elem_size=64, elem_step=TW)

                        if debug and l == 0 and t == 0 and gi == 0:
                            dbgm_sb = wpool.tile([P, 8 * HC], f32, tag="dbgm")
                            nc.vector.tensor_copy(
                                out=dbgm_sb[:].rearrange("p (a b) -> p a b", a=8),
                                in_=msg[:, :, 0:HC])
                            nc.sync.dma_start(out=DBGM[:], in_=dbgm_sb[:])
                        # ee = exp(leaky_relu(es + ed))
                        ee = wpool.tile([P, GMAX, 4], f32, tag="ee")
                        tmp = wpool.tile([P, GMAX, 4], f32, tag="tmp")
                        nc.vector.tensor_add(out=ee[:, 0:g, :],
                                             in0=msg[:, 0:g, HC:HC + 4],
                                             in1=edb[:, 0:g, 4:8])
                        nc.vector.tensor_scalar_mul(tmp[:, 0:g, :],
                                                    ee[:, 0:g, :], NEG_SLOPE)
                        nc.vector.tensor_max(out=ee[:, 0:g, :],
                                             in0=ee[:, 0:g, :],
                                             in1=tmp[:, 0:g, :])
                        if no_exp:
                            nc.vector.memset(ee[:, 0:g, :], 1.0)
                        else:
                            nc.scalar.activation(
                                out=ee[:, 0:g, :], in_=ee[:, 0:g, :],
                                func=mybir.ActivationFunctionType.Exp)

                        for j in range(g):
                            jj = ga + j
                            S = spool.tile([P, P], f32, tag="S")
                            nc.vector.tensor_tensor(
                                out=S[:],
                                in0=dloc_sb[:, t * cpt + jj:t * cpt + jj + 1]
                                    .to_broadcast([P, P]),
                                in1=iota[:],
                                op=mybir.AluOpType.is_equal)
                            mj = msg[:, j, 0:HC].rearrange("p (h c) -> p h c",
                                                           h=HEADS)
                            nc.vector.tensor_mul(
                                out=mj, in0=mj,
                                in1=ee[:, j, :].to_broadcast([P, HEADS, HID]))
                            first = (jj == 0)
                            last = (jj == agg_chunks - 1)
                            nc.tensor.matmul(out=psn[:], lhsT=S[:],
                                             rhs=msg[:, j, 0:HC],
                                             start=first, stop=last)
                            nc.tensor.matmul(out=psd[:], lhsT=S[:],
                                             rhs=ee[:, j, :],
                                             start=first, stop=last)

                    if debug and l == 0 and t == 0:
                        dbgp_sb = wpool.tile([P, 260], f32, tag="dbgp")
                        nc.vector.tensor_copy(out=dbgp_sb[:, 0:HC], in_=psn[:])
                        nc.vector.tensor_copy(out=dbgp_sb[:, HC:260], in_=psd[:])
                        nc.sync.dma_start(out=DBGP[:], in_=dbgp_sb[:])
                    # epilogue: divide, bias, gelu
                    rcp = wpool.tile([P, 4], f32, tag="rcp")
                    nc.vector.reciprocal(out=rcp[:], in_=psd[:])
                    xn = XN_sb[:, t * HC:(t + 1) * HC]
                    nc.vector.tensor_mul(
                        out=xn.rearrange("p (h c) -> p h c", h=HEADS),
                        in0=psn[:].rearrange("p (h c) -> p h c", h=HEADS),
                        in1=rcp[:].to_broadcast([P, HEADS, HID]))
                    nc.vector.tensor_add(out=xn, in0=xn,
                                         in1=BB_sb[:, l * HC:(l + 1) * HC])
                    nc.scalar.activation(out=xn, in_=xn,
                                         func=mybir.ActivationFunctionType.Gelu)

                if debug and l == n_layers - 1:
                    dbgx_sb = wpool.tile([P, TPC * HC], f32, tag="dbgx")
                    nc.vector.tensor_copy(out=dbgx_sb[:], in_=XN_sb[:])
                    nc.sync.dma_start(out=DBGX[:], in_=dbgx_sb[:])

                # --- phase D: transpose for next matmuls ------------------
                for t in range(TPC):
                    for k in range(2):
                        pstr = psTpool.tile([P, P], f32, tag="psT")
                        nc.tensor.transpose(
                            out=pstr[:],
                            in_=XN_sb[:, t * HC + k * P:t * HC + (k + 1) * P],
                            identity=ident[:])
                        nc.vector.tensor_copy(
                            out=XT_sb[:, k, t * P:(t + 1) * P], in_=pstr[:])

            # --- final projection ----------------------------------------
            for t in range(TPC):
                psp = psTpool.tile([P, OUT_C], f32, tag="psT")
                nc.tensor.matmul(out=psp[:], lhsT=XT_sb[:, 0, t * P:(t + 1) * P],
                                 rhs=RP_sb[:, 0, :], start=True, stop=False)
                nc.tensor.matmul(out=psp[:], lhsT=XT_sb[:, 1, t * P:(t + 1) * P],
                                 rhs=RP_sb[:, 1, :], start=False, stop=True)
                yt = wpool.tile([P, OUT_C], f32, tag="yt")
                nc.vector.tensor_add(out=yt[:], in0=psp[:], in1=BP_sb[:])
                nc.sync.dma_start(out=Y[t * P:(t + 1) * P, :], in_=yt[:])

    nc.compile()
    _PROGRAM_CACHE[cache_key] = nc
    return nc


# ----------------------------------------------------------------------------
# entry point
# ----------------------------------------------------------------------------

def _make_in_maps(inputs, perm, cpt, per_core):
    x = np.asarray(inputs["x"], np.float32)
    x_pad = np.zeros((NPAD, IN_C), np.float32)
    x_pad[perm] = x
    xT_all = np.ascontiguousarray(x_pad.T)  # [128, 20480]

    Rs, BBa, BPa, RPa = _pack_weights(inputs)

    in_maps = []
    for c in range(NCORES):
        sidx, didx, dl = per_core[c]
        in_maps.append({
            "xT": np.ascontiguousarray(
                xT_all[:, c * TPC * P:(c + 1) * TPC * P]),
            "R0": Rs[0], "R1": Rs[1], "R2": Rs[2],
            "RP": RPa, "BB": BBa, "BP": BPa,
            "SIDX": sidx, "DIDX": didx, "DLOC": dl,
        })
    return in_maps


def kernel(**inputs) -> np.ndarray:
    edge_index = np.asarray(inputs["edge_index"])
    perm, cpt, per_core = _preprocess(edge_index)
    nc = _build_program(cpt)
    in_maps = _make_in_maps(inputs, perm, cpt, per_core)
    res = run_bass_kernel_spmd(nc, in_maps, core_ids=list(range(NCORES)))
    y_new = np.concatenate([res.results[c]["Y"] for c in range(NCORES)], axis=0)
    return np.ascontiguousarray(y_new[perm]).astype(np.float32)
